# revision 20
# baseline (speedup 1.0000x reference)
"""Trainium2 Bass kernel for single-head cross-attention.

Reference computation (B=4, Sq=Skv=2048, D=1024, fp32):
    Q = query @ Wq + bq ; K = key @ Wk + bk ; V = value @ Wv + bv
    out = softmax(Q K^T / sqrt(D)) V @ Wo + bo

Weight folding (host, exact in fp32): softmax((qWq + bq)(kWk + bk)^T) equals
softmax(q M k^T + 1 x d^T) with M = Wq Wk^T and d = (k Wk) bq, because the
per-query-row term (qWq) bk and the constant bq.bk shift every score in a row
equally and cancel in softmax. Likewise (A (vWv + bv) Wo)/sums + bo =
(A (v N))/sums + bo2 with N = Wv Wo, bo2 = bv Wo + bo. So the device computes
only:
    Q'^T[e,q] = M^T @ qT          (lhsT=M,    rhs=qT)
    V'[kv,f]  = vT.T @ N          (lhsT=vT,   rhs=N)    own kv half -> AllGather
    S^T[kv,q] = k @ Q'^T          (lhsT=kT,   rhs=Q'^T) kT is the RAW key input
    A^T       = exp(S^T/32 + dsc) (dsc = d/32 as per-kv-partition bias)
    sums[q,1] = A @ ones          (lhsT=A^T,  rhs=ones)
    out[q,f]  = (A @ V') * (1/sums) + bo2   (lhsT=A^T, rhs=V')

Sharding: 8 shards = (batch b in 0..3) x (query half h in 0..1); core
c = 2*b + h computes output rows [h*1024,(h+1)*1024) of batch b. Each core
projects only its kv-half of V' and the pair exchanges halves with one
AllGather, which hides under Q' projection + both score blocks (~95us slack).
Raw keys need no projection at all and stream straight from HBM.
"""

import sys

if "/opt/trn_rl_repo" not in sys.path:
    sys.path.insert(0, "/opt/trn_rl_repo")

from contextlib import ExitStack

import ml_dtypes
import numpy as np

import concourse.bass as bass
import concourse.mybir as mybir
import concourse.tile as tile
from concourse import bacc
from concourse.bass_utils import run_bass_kernel_spmd

B, SQ, SKV, D = 4, 2048, 2048, 1024
NCORES = 8
QL = SQ // 2  # local query rows per core
KVH = SKV // 2  # own kv half per core
P = 128
DC = D // P  # feature chunks (8)
KVC = SKV // P  # kv chunks (16)
KVHC = KVH // P  # own-half kv chunks (8)
N5 = 512
F32 = mybir.dt.float32
CDT = mybir.dt.bfloat16  # on-device compute dtype for matmul operands
F8 = mybir.dt.float8e4  # scores matmul runs double-pumped e4m3
NP_CDT = ml_dtypes.bfloat16
NP_F8 = ml_dtypes.float8_e4m3
SCALE = 1.0 / 32.0  # 1/sqrt(D)
QP8_SCALE = 32.0  # Q' stored in e4m3 at 32x (sigma ~13, max 240)
K8_SCALE = 16.0  # raw keys stored in e4m3 at 16x (sigma 16)
DR = mybir.MatmulPerfMode.DoubleRow

AF = mybir.ActivationFunctionType
GROUPS = [[0, 1], [2, 3], [4, 5], [6, 7]]


def _build_tile(ctx: ExitStack, tc, aps, dram):
    nc = tc.nc
    qT, kT, vT, m, n, dsc, bo2, out = aps
    vg_half, vg_full = dram

    weights = ctx.enter_context(tc.tile_pool(name="weights", bufs=1))
    big = ctx.enter_context(tc.tile_pool(name="big", bufs=1))
    attn_pool = ctx.enter_context(tc.tile_pool(name="attn", bufs=2))
    evac = ctx.enter_context(tc.tile_pool(name="evac", bufs=4))
    psum = ctx.enter_context(tc.tile_pool(name="psum", bufs=4, space="PSUM"))
    psum_s = ctx.enter_context(tc.tile_pool(name="psum_s", bufs=2, space="PSUM"))

    qT_r = qT.rearrange("(c p) n -> p c n", p=P)
    kT_r = kT.rearrange("(c p) n -> p c n", p=P)
    vT_r = vT.rearrange("(c p) n -> p c n", p=P)
    n_r = n.rearrange("(c p) e -> p c e", p=P)
    m_r = m.rearrange("(c p) e -> p c e", p=P)

    # All inputs ride the SP ring as few, large DMAs; the FIFO delivers them
    # in exactly consumption order (n+v for V'proj, m+q for Q'proj, then kS
    # for scores). Each dma_start costs ~0.65us of sequencer issue time, and
    # a big DMA issued early starves later ones, so order is everything.
    nS = weights.tile([P, DC, D], CDT, tag="nS")
    vS = weights.tile([P, DC, KVH], CDT, tag="vS")
    # First-phase operands split across BOTH rings so they dispatch
    # concurrently (the ACT ring is otherwise idle until the dumps).
    nc.scalar.dma_start(out=nS[:, 0:4, :], in_=n_r[:, 0:4, :])
    nc.scalar.dma_start(out=vS[:, :, 0:N5], in_=vT_r[:, :, 0:N5])
    nc.sync.dma_start(out=nS[:, 4:8, :], in_=n_r[:, 4:8, :])
    nc.sync.dma_start(out=vS[:, :, N5:KVH], in_=vT_r[:, :, N5:KVH])
    mS = weights.tile([P, DC, D], CDT, tag="mS")
    qS = weights.tile([P, DC, QL], CDT, tag="qS")
    nc.sync.dma_start(out=mS, in_=m_r)
    nc.sync.dma_start(out=qS, in_=qT_r)
    kS = big.tile([P, DC, SKV], F8, tag="kS")
    nc.sync.dma_start(out=kS[:, :, 0:KVH], in_=kT_r[:, :, 0:KVH])
    nc.sync.dma_start(out=kS[:, :, KVH:SKV], in_=kT_r[:, :, KVH:SKV])
    dsc_s = weights.tile([P, KVC], F32, tag="dsc")
    nc.sync.dma_start(out=dsc_s, in_=dsc.rearrange("(c p) -> p c", p=P))
    bo2_s = weights.tile([P, D], F32, tag="bo2")
    bo2_bcast = bass.AP(tensor=bo2.tensor, offset=bo2.offset, ap=[[0, P], bo2.ap[0]])
    nc.sync.dma_start(out=bo2_s, in_=bo2_bcast)

    # ---- V' projection, own kv half -> vO[:, 0:KVHC, :] -> dump -> AllGather --
    # The gather is split in two so the first half launches as soon as kv
    # chunks 0-3 are projected (~15us earlier); the reload rides the SP ring
    # because a DMA waiting in a ring queue blocks everything behind it, and
    # the ACT ring must keep flowing (Q' evacs + EXPs).
    vO = big.tile([P, KVC, D], CDT, tag="vO")  # V': [kv%128, kv//128, f]
    for j in range(KVH // N5):
        x_in = vS[:, :, j * N5 : (j + 1) * N5]
        for sub in range(N5 // P):
            c = j * (N5 // P) + sub
            for nv in range(D // N5):
                ps = psum.tile([P, N5], F32, tag="mm")
                for dc in range(DC):
                    nc.tensor.matmul(
                        ps,
                        lhsT=x_in[:, dc, sub * P : (sub + 1) * P],
                        rhs=nS[:, dc, nv * N5 : (nv + 1) * N5],
                        start=(dc == 0),
                        stop=(dc == DC - 1),
                    )
                nc.vector.tensor_copy(out=vO[:, c, nv * N5 : (nv + 1) * N5], in_=ps)
            # Dump each finished 128-kv-row chunk so the gather starts early.
            nc.scalar.dma_start(out=vg_half[j][:, sub, :], in_=vO[:, c, :])
        nc.gpsimd.collective_compute(
            "AllGather",
            mybir.AluOpType.bypass,
            replica_groups=GROUPS,
            ins=[vg_half[j][:]],
            outs=[vg_full[j][:]],
        )
    # Reload rewrites ALL of vO in rank order (rank g owns kv half g).
    for j in range(2):
        for g in range(2):
            nc.sync.dma_start(
                out=vO[:, g * KVHC + j * 4 : g * KVHC + (j + 1) * 4, :],
                in_=vg_full[j][g, :, :, :],
            )

    # ---- Q' projection (overlaps the collective) -----------------------------
    ones = weights.tile([P, 1], CDT, tag="ones")
    nc.vector.memset(ones, 1.0)

    qTo = big.tile([P, DC, QL], F8, tag="qTo")  # Q'^T: [e%128, e//128, q]
    for j in range(QL // N5):
        x_in = qS[:, :, j * N5 : (j + 1) * N5]
        for ec in range(DC):
            ps = psum.tile([P, N5], F32, tag="mm")
            for dc in range(DC):
                nc.tensor.matmul(
                    ps,
                    lhsT=mS[:, dc, ec * P : (ec + 1) * P],
                    rhs=x_in[:, dc, :],
                    start=(dc == 0),
                    stop=(dc == DC - 1),
                )
            nc.scalar.activation(
                out=qTo[:, ec, j * N5 : (j + 1) * N5],
                in_=ps,
                func=AF.Identity,
                scale=QP8_SCALE,
            )

    # ---- attention: scores+sums for both 512-query blocks first, then the
    # A@V' passes, so the V' gather has the whole scores span to complete. ----
    blocks = []
    for qb in range(QL // N5):
        attnT = attn_pool.tile([P, KVC, N5], CDT, tag="attnT")
        for c in range(KVC):
            ps = psum.tile([P, N5], F32, tag="mm")
            for ep in range(DC // 2):
                nc.tensor.matmul(
                    ps,
                    lhsT=kS[:, 2 * ep : 2 * ep + 2, c * P : (c + 1) * P],
                    rhs=qTo[:, 2 * ep : 2 * ep + 2, qb * N5 : (qb + 1) * N5],
                    start=(ep == 0),
                    stop=(ep == DC // 2 - 1),
                    perf_mode=DR,
                )
            nc.scalar.activation(
                out=attnT[:, c, :],
                in_=ps,
                func=AF.Exp,
                bias=dsc_s[:, c : c + 1],
                scale=SCALE / (QP8_SCALE * K8_SCALE),
            )

        # softmax denominators: sums[q,1] = A^T.T @ ones, accumulated over kv
        ps_sum = psum_s.tile([P, N5 // P], F32, tag="sums")
        for s in range(N5 // P):
            for c in range(KVC):
                nc.tensor.matmul(
                    ps_sum[:, s : s + 1],
                    lhsT=attnT[:, c, s * P : (s + 1) * P],
                    rhs=ones[:, :1],
                    start=(c == 0),
                    stop=(c == KVC - 1),
                )
        r_s = evac.tile([P, N5 // P], F32, tag="recip")
        nc.vector.reciprocal(r_s, ps_sum)
        blocks.append((attnT, r_s))

    for qb in range(QL // N5):
        attnT, r_s = blocks[qb]
        for s in range(N5 // P):
            for nf in range(D // N5):
                ps = psum.tile([P, N5], F32, tag="mm")
                for c in range(KVC):
                    nc.tensor.matmul(
                        ps,
                        lhsT=attnT[:, c, s * P : (s + 1) * P],
                        rhs=vO[:, c, nf * N5 : (nf + 1) * N5],
                        start=(c == 0),
                        stop=(c == KVC - 1),
                    )
                fin = evac.tile([P, N5], F32, tag="fin")
                nc.vector.scalar_tensor_tensor(
                    out=fin,
                    in0=ps,
                    scalar=r_s[:, s : s + 1],
                    in1=bo2_s[:, nf * N5 : (nf + 1) * N5],
                    op0=mybir.AluOpType.mult,
                    op1=mybir.AluOpType.add,
                )
                row0 = qb * N5 + s * P
                nc.sync.dma_start(
                    out=out[row0 : row0 + P, nf * N5 : (nf + 1) * N5], in_=fin
                )


def build_program():
    nc = bacc.Bacc(
        "TRN2", target_bir_lowering=False, debug=False, num_devices=NCORES
    )
    qT = nc.dram_tensor("qT", [D, QL], CDT, kind="ExternalInput").ap()
    kT = nc.dram_tensor("kT", [D, SKV], F8, kind="ExternalInput").ap()
    vT = nc.dram_tensor("vT", [D, KVH], CDT, kind="ExternalInput").ap()
    m = nc.dram_tensor("m", [D, D], CDT, kind="ExternalInput").ap()
    n = nc.dram_tensor("n", [D, D], CDT, kind="ExternalInput").ap()
    dsc = nc.dram_tensor("dsc", [SKV], F32, kind="ExternalInput").ap()
    bo2 = nc.dram_tensor("bo2", [D], F32, kind="ExternalInput").ap()
    out = nc.dram_tensor("out", [QL, D], F32, kind="ExternalOutput").ap()

    vg_half = [
        nc.dram_tensor(f"vg_half{j}", [P, KVHC // 2, D], CDT).ap() for j in range(2)
    ]
    vg_full = [
        nc.dram_tensor(f"vg_full{j}", [2, P, KVHC // 2, D], CDT).ap()
        for j in range(2)
    ]
    with tile.TileContext(nc) as tc:
        with ExitStack() as ctx:
            _build_tile(
                ctx,
                tc,
                (qT, kT, vT, m, n, dsc, bo2, out),
                (vg_half, vg_full),
            )
    nc.compile()
    return nc


def prep_in_maps(query, key, value, Wq, bq, Wk, bk, Wv, bv, Wo, bo):
    """Host-side shard prep: fold weights, slice, transpose to feature-major."""
    query = np.asarray(query, np.float32)
    key = np.asarray(key, np.float32)
    value = np.asarray(value, np.float32)
    Wq = np.asarray(Wq, np.float32)
    Wk = np.asarray(Wk, np.float32)
    Wv = np.asarray(Wv, np.float32)
    Wo = np.asarray(Wo, np.float32)
    bq = np.asarray(bq, np.float32)
    bv = np.asarray(bv, np.float32)
    bo = np.asarray(bo, np.float32)

    M = (Wq @ Wk.T).astype(NP_CDT)
    N = (Wv @ Wo).astype(NP_CDT)
    bo2 = bv @ Wo + bo
    h_vec = Wk @ bq  # per-kv score bias direction (cancels nothing: kv-varying)
    shared = {"m": M, "n": N, "bo2": bo2}
    in_maps = []
    for b in range(B):
        kTb = np.ascontiguousarray(key[b].T * np.float32(K8_SCALE)).astype(NP_F8)
        dsc_b = (key[b] @ h_vec) * np.float32(SCALE)
        for h in range(2):
            qTb = np.ascontiguousarray(query[b, h * QL : (h + 1) * QL].T).astype(
                NP_CDT
            )
            vTb = np.ascontiguousarray(value[b, h * KVH : (h + 1) * KVH].T).astype(
                NP_CDT
            )
            in_maps.append(
                {
                    "qT": qTb,
                    "kT": kTb,
                    "vT": vTb,
                    "dsc": dsc_b,
                    **shared,
                }
            )
    return in_maps


_NC_CACHE = None


def _get_nc():
    global _NC_CACHE
    if _NC_CACHE is None:
        _NC_CACHE = build_program()
    return _NC_CACHE


def run(inputs, **run_kwargs):
    nc = _get_nc()
    in_maps = prep_in_maps(**inputs)
    res = run_bass_kernel_spmd(nc, in_maps, core_ids=list(range(NCORES)), **run_kwargs)
    out = np.empty((B, SQ, D), np.float32)
    for b in range(B):
        for h in range(2):
            out[b, h * QL : (h + 1) * QL] = res.results[2 * b + h]["out"]
    return out, res


def kernel(query, key, value, Wq, bq, Wk, bk, Wv, bv, Wo, bo):
    out, _ = run(
        dict(
            query=query, key=key, value=value, Wq=Wq, bq=bq, Wk=Wk, bk=bk,
            Wv=Wv, bv=bv, Wo=Wo, bo=bo,
        )
    )
    return out


if __name__ == "__main__":
    rng = np.random.default_rng(0)
    ins = {
        "query": rng.standard_normal((B, SQ, D), dtype=np.float32),
        "key": rng.standard_normal((B, SQ, D), dtype=np.float32),
        "value": rng.standard_normal((B, SQ, D), dtype=np.float32),
        "Wq": (rng.standard_normal((D, D), dtype=np.float32) * 0.02),
        "bq": np.zeros(D, np.float32),
        "Wk": (rng.standard_normal((D, D), dtype=np.float32) * 0.02),
        "bk": np.zeros(D, np.float32),
        "Wv": (rng.standard_normal((D, D), dtype=np.float32) * 0.02),
        "bv": np.zeros(D, np.float32),
        "Wo": (rng.standard_normal((D, D), dtype=np.float32) * 0.02),
        "bo": np.zeros(D, np.float32),
    }
    out = kernel(**ins)
    print("kernel ran, out shape", out.shape)


# revision 21
# speedup vs baseline: 1.0898x; 1.0898x over previous
"""Trainium2 Bass kernel for single-head cross-attention.

Reference computation (B=4, Sq=Skv=2048, D=1024, fp32):
    Q = query @ Wq + bq ; K = key @ Wk + bk ; V = value @ Wv + bv
    out = softmax(Q K^T / sqrt(D)) V @ Wo + bo

Weight folding (host, exact in fp32): softmax((qWq + bq)(kWk + bk)^T) equals
softmax(q M k^T + 1 x d^T) with M = Wq Wk^T and d = (k Wk) bq, because the
per-query-row term (qWq) bk and the constant bq.bk shift every score in a row
equally and cancel in softmax. Likewise (A (vWv + bv) Wo)/sums + bo =
(A (v N))/sums + bo2 with N = Wv Wo, bo2 = bv Wo + bo. So the device computes
only:
    Q'^T[e,q] = M^T @ qT          (lhsT=M,    rhs=qT)
    V'[kv,f]  = vT.T @ N          (lhsT=vT,   rhs=N)    own kv half -> AllGather
    S^T[kv,q] = k @ Q'^T          (lhsT=kT,   rhs=Q'^T) kT is the RAW key input
    A^T       = exp(S^T/32 + dsc) (dsc = d/32 as per-kv-partition bias)
    sums[q,1] = A @ ones          (lhsT=A^T,  rhs=ones)
    out[q,f]  = (A @ V') * (1/sums) + bo2   (lhsT=A^T, rhs=V')

Sharding: 8 shards = (batch b in 0..3) x (query half h in 0..1); core
c = 2*b + h computes output rows [h*1024,(h+1)*1024) of batch b. Each core
projects only its kv-half of V' and the pair exchanges halves with one
AllGather, which hides under Q' projection + both score blocks (~95us slack).
Raw keys need no projection at all and stream straight from HBM.
"""

import sys

if "/opt/trn_rl_repo" not in sys.path:
    sys.path.insert(0, "/opt/trn_rl_repo")

from contextlib import ExitStack

import ml_dtypes
import numpy as np

import concourse.bass as bass
import concourse.mybir as mybir
import concourse.tile as tile
from concourse import bacc
from concourse.bass_utils import run_bass_kernel_spmd

B, SQ, SKV, D = 4, 2048, 2048, 1024
NCORES = 8
QL = SQ // 2  # local query rows per core
KVH = SKV // 2  # own kv half per core
P = 128
DC = D // P  # feature chunks (8)
KVC = SKV // P  # kv chunks (16)
KVHC = KVH // P  # own-half kv chunks (8)
N5 = 512
F32 = mybir.dt.float32
CDT = mybir.dt.bfloat16  # on-device compute dtype for matmul operands
F8 = mybir.dt.float8e4  # scores matmul runs double-pumped e4m3
NP_CDT = ml_dtypes.bfloat16
NP_F8 = ml_dtypes.float8_e4m3
SCALE = 1.0 / 32.0  # 1/sqrt(D)
QP8_SCALE = 32.0  # Q' stored in e4m3 at 32x (sigma ~13, max 240)
K8_SCALE = 16.0  # raw keys stored in e4m3 at 16x (sigma 16)
DR = mybir.MatmulPerfMode.DoubleRow

AF = mybir.ActivationFunctionType
GROUPS = [[0, 1], [2, 3], [4, 5], [6, 7]]


def _build_tile(ctx: ExitStack, tc, aps, dram):
    nc = tc.nc
    qT, kT, vT, m, n, dsc, bo2, out = aps
    vg_half, vg_full = dram

    weights = ctx.enter_context(tc.tile_pool(name="weights", bufs=1))
    big = ctx.enter_context(tc.tile_pool(name="big", bufs=1))
    attn_pool = ctx.enter_context(tc.tile_pool(name="attn", bufs=2))
    evac = ctx.enter_context(tc.tile_pool(name="evac", bufs=4))
    psum = ctx.enter_context(tc.tile_pool(name="psum", bufs=4, space="PSUM"))
    psum_s = ctx.enter_context(tc.tile_pool(name="psum_s", bufs=2, space="PSUM"))

    qT_r = qT.rearrange("(c p) n -> p c n", p=P)
    kT_r = kT.rearrange("(c p) n -> p c n", p=P)
    vT_r = vT.rearrange("(c p) n -> p c n", p=P)
    n_r = n.rearrange("(c p) e -> p c e", p=P)
    m_r = m.rearrange("(c p) e -> p c e", p=P)

    # All inputs ride the SP ring as few, large DMAs; the FIFO delivers them
    # in exactly consumption order (n+v for V'proj, m+q for Q'proj, then kS
    # for scores). Each dma_start costs ~0.65us of sequencer issue time, and
    # a big DMA issued early starves later ones, so order is everything.
    nS = weights.tile([P, DC, D], CDT, tag="nS")
    vS = weights.tile([P, DC, KVH], CDT, tag="vS")
    nc.sync.dma_start(out=nS[:, 0:4, :], in_=n_r[:, 0:4, :])
    nc.sync.dma_start(out=vS[:, :, 0:N5], in_=vT_r[:, :, 0:N5])
    nc.sync.dma_start(out=nS[:, 4:8, :], in_=n_r[:, 4:8, :])
    nc.sync.dma_start(out=vS[:, :, N5:KVH], in_=vT_r[:, :, N5:KVH])
    mS = weights.tile([P, DC, D], CDT, tag="mS")
    qS = weights.tile([P, DC, QL], CDT, tag="qS")
    nc.sync.dma_start(out=mS, in_=m_r)
    nc.sync.dma_start(out=qS, in_=qT_r)
    kS = big.tile([P, DC, SKV], F8, tag="kS")
    nc.sync.dma_start(out=kS[:, :, 0:KVH], in_=kT_r[:, :, 0:KVH])
    nc.sync.dma_start(out=kS[:, :, KVH:SKV], in_=kT_r[:, :, KVH:SKV])
    dsc_s = weights.tile([P, KVC], F32, tag="dsc")
    nc.sync.dma_start(out=dsc_s, in_=dsc.rearrange("(c p) -> p c", p=P))
    bo2_s = weights.tile([P, D], F32, tag="bo2")
    bo2_bcast = bass.AP(tensor=bo2.tensor, offset=bo2.offset, ap=[[0, P], bo2.ap[0]])
    nc.sync.dma_start(out=bo2_s, in_=bo2_bcast)

    # ---- V' projection, own kv half -> vO[:, 0:KVHC, :] -> dump -> AllGather --
    # The gather is split in two so the first half launches as soon as kv
    # chunks 0-3 are projected (~15us earlier); the reload rides the SP ring
    # because a DMA waiting in a ring queue blocks everything behind it, and
    # the ACT ring must keep flowing (Q' evacs + EXPs).
    vO = big.tile([P, KVC, D], CDT, tag="vO")  # V': [kv%128, kv//128, f]
    for j in range(KVH // N5):
        x_in = vS[:, :, j * N5 : (j + 1) * N5]
        for sub in range(N5 // P):
            c = j * (N5 // P) + sub
            for nv in range(D // N5):
                ps = psum.tile([P, N5], F32, tag="mm")
                for dc in range(DC):
                    nc.tensor.matmul(
                        ps,
                        lhsT=x_in[:, dc, sub * P : (sub + 1) * P],
                        rhs=nS[:, dc, nv * N5 : (nv + 1) * N5],
                        start=(dc == 0),
                        stop=(dc == DC - 1),
                    )
                nc.vector.tensor_copy(out=vO[:, c, nv * N5 : (nv + 1) * N5], in_=ps)
            # Dump each finished 128-kv-row chunk so the gather starts early.
            nc.scalar.dma_start(out=vg_half[j][:, sub, :], in_=vO[:, c, :])
        nc.gpsimd.collective_compute(
            "AllGather",
            mybir.AluOpType.bypass,
            replica_groups=GROUPS,
            ins=[vg_half[j][:]],
            outs=[vg_full[j][:]],
        )
    # Reload rewrites ALL of vO in rank order (rank g owns kv half g).
    for j in range(2):
        for g in range(2):
            nc.sync.dma_start(
                out=vO[:, g * KVHC + j * 4 : g * KVHC + (j + 1) * 4, :],
                in_=vg_full[j][g, :, :, :],
            )

    # ---- Q' projection (overlaps the collective) -----------------------------
    ones = weights.tile([P, 1], CDT, tag="ones")
    nc.vector.memset(ones, 1.0)

    qTo = big.tile([P, DC, QL], F8, tag="qTo")  # Q'^T: [e%128, e//128, q]
    for j in range(QL // N5):
        x_in = qS[:, :, j * N5 : (j + 1) * N5]
        for ec in range(DC):
            ps = psum.tile([P, N5], F32, tag="mm")
            for dc in range(DC):
                nc.tensor.matmul(
                    ps,
                    lhsT=mS[:, dc, ec * P : (ec + 1) * P],
                    rhs=x_in[:, dc, :],
                    start=(dc == 0),
                    stop=(dc == DC - 1),
                )
            nc.scalar.activation(
                out=qTo[:, ec, j * N5 : (j + 1) * N5],
                in_=ps,
                func=AF.Identity,
                scale=QP8_SCALE,
            )

    # ---- attention: scores+sums for both 512-query blocks first, then the
    # A@V' passes, so the V' gather has the whole scores span to complete. ----
    blocks = []
    for qb in range(QL // N5):
        attnT = attn_pool.tile([P, KVC, N5], CDT, tag="attnT")
        for c in range(KVC):
            ps = psum.tile([P, N5], F32, tag="mm")
            for ep in range(DC // 2):
                nc.tensor.matmul(
                    ps,
                    lhsT=kS[:, 2 * ep : 2 * ep + 2, c * P : (c + 1) * P],
                    rhs=qTo[:, 2 * ep : 2 * ep + 2, qb * N5 : (qb + 1) * N5],
                    start=(ep == 0),
                    stop=(ep == DC // 2 - 1),
                    perf_mode=DR,
                )
            nc.scalar.activation(
                out=attnT[:, c, :],
                in_=ps,
                func=AF.Exp,
                bias=dsc_s[:, c : c + 1],
                scale=SCALE / (QP8_SCALE * K8_SCALE),
            )

        # softmax denominators: sums[q,1] = A^T.T @ ones, accumulated over kv
        ps_sum = psum_s.tile([P, N5 // P], F32, tag="sums")
        for s in range(N5 // P):
            for c in range(KVC):
                nc.tensor.matmul(
                    ps_sum[:, s : s + 1],
                    lhsT=attnT[:, c, s * P : (s + 1) * P],
                    rhs=ones[:, :1],
                    start=(c == 0),
                    stop=(c == KVC - 1),
                )
        r_s = evac.tile([P, N5 // P], F32, tag="recip")
        nc.vector.reciprocal(r_s, ps_sum)
        blocks.append((attnT, r_s))

    for qb in range(QL // N5):
        attnT, r_s = blocks[qb]
        for s in range(N5 // P):
            for nf in range(D // N5):
                ps = psum.tile([P, N5], F32, tag="mm")
                for c in range(KVC):
                    nc.tensor.matmul(
                        ps,
                        lhsT=attnT[:, c, s * P : (s + 1) * P],
                        rhs=vO[:, c, nf * N5 : (nf + 1) * N5],
                        start=(c == 0),
                        stop=(c == KVC - 1),
                    )
                fin = evac.tile([P, N5], F32, tag="fin")
                nc.vector.scalar_tensor_tensor(
                    out=fin,
                    in0=ps,
                    scalar=r_s[:, s : s + 1],
                    in1=bo2_s[:, nf * N5 : (nf + 1) * N5],
                    op0=mybir.AluOpType.mult,
                    op1=mybir.AluOpType.add,
                )
                row0 = qb * N5 + s * P
                nc.sync.dma_start(
                    out=out[row0 : row0 + P, nf * N5 : (nf + 1) * N5], in_=fin
                )


def build_program():
    nc = bacc.Bacc(
        "TRN2", target_bir_lowering=False, debug=False, num_devices=NCORES
    )
    qT = nc.dram_tensor("qT", [D, QL], CDT, kind="ExternalInput").ap()
    kT = nc.dram_tensor("kT", [D, SKV], F8, kind="ExternalInput").ap()
    vT = nc.dram_tensor("vT", [D, KVH], CDT, kind="ExternalInput").ap()
    m = nc.dram_tensor("m", [D, D], CDT, kind="ExternalInput").ap()
    n = nc.dram_tensor("n", [D, D], CDT, kind="ExternalInput").ap()
    dsc = nc.dram_tensor("dsc", [SKV], F32, kind="ExternalInput").ap()
    bo2 = nc.dram_tensor("bo2", [D], F32, kind="ExternalInput").ap()
    out = nc.dram_tensor("out", [QL, D], F32, kind="ExternalOutput").ap()

    vg_half = [
        nc.dram_tensor(f"vg_half{j}", [P, KVHC // 2, D], CDT).ap() for j in range(2)
    ]
    vg_full = [
        nc.dram_tensor(f"vg_full{j}", [2, P, KVHC // 2, D], CDT).ap()
        for j in range(2)
    ]
    with tile.TileContext(nc) as tc:
        with ExitStack() as ctx:
            _build_tile(
                ctx,
                tc,
                (qT, kT, vT, m, n, dsc, bo2, out),
                (vg_half, vg_full),
            )
    nc.compile()
    return nc


def prep_in_maps(query, key, value, Wq, bq, Wk, bk, Wv, bv, Wo, bo):
    """Host-side shard prep: fold weights, slice, transpose to feature-major."""
    query = np.asarray(query, np.float32)
    key = np.asarray(key, np.float32)
    value = np.asarray(value, np.float32)
    Wq = np.asarray(Wq, np.float32)
    Wk = np.asarray(Wk, np.float32)
    Wv = np.asarray(Wv, np.float32)
    Wo = np.asarray(Wo, np.float32)
    bq = np.asarray(bq, np.float32)
    bv = np.asarray(bv, np.float32)
    bo = np.asarray(bo, np.float32)

    M = (Wq @ Wk.T).astype(NP_CDT)
    N = (Wv @ Wo).astype(NP_CDT)
    bo2 = bv @ Wo + bo
    h_vec = Wk @ bq  # per-kv score bias direction (cancels nothing: kv-varying)
    shared = {"m": M, "n": N, "bo2": bo2}
    in_maps = []
    for b in range(B):
        kTb = np.ascontiguousarray(key[b].T * np.float32(K8_SCALE)).astype(NP_F8)
        dsc_b = (key[b] @ h_vec) * np.float32(SCALE)
        for h in range(2):
            qTb = np.ascontiguousarray(query[b, h * QL : (h + 1) * QL].T).astype(
                NP_CDT
            )
            vTb = np.ascontiguousarray(value[b, h * KVH : (h + 1) * KVH].T).astype(
                NP_CDT
            )
            in_maps.append(
                {
                    "qT": qTb,
                    "kT": kTb,
                    "vT": vTb,
                    "dsc": dsc_b,
                    **shared,
                }
            )
    return in_maps


_NC_CACHE = None


def _get_nc():
    global _NC_CACHE
    if _NC_CACHE is None:
        _NC_CACHE = build_program()
    return _NC_CACHE


def run(inputs, **run_kwargs):
    nc = _get_nc()
    in_maps = prep_in_maps(**inputs)
    res = run_bass_kernel_spmd(nc, in_maps, core_ids=list(range(NCORES)), **run_kwargs)
    out = np.empty((B, SQ, D), np.float32)
    for b in range(B):
        for h in range(2):
            out[b, h * QL : (h + 1) * QL] = res.results[2 * b + h]["out"]
    return out, res


def kernel(query, key, value, Wq, bq, Wk, bk, Wv, bv, Wo, bo):
    out, _ = run(
        dict(
            query=query, key=key, value=value, Wq=Wq, bq=bq, Wk=Wk, bk=bk,
            Wv=Wv, bv=bv, Wo=Wo, bo=bo,
        )
    )
    return out


if __name__ == "__main__":
    rng = np.random.default_rng(0)
    ins = {
        "query": rng.standard_normal((B, SQ, D), dtype=np.float32),
        "key": rng.standard_normal((B, SQ, D), dtype=np.float32),
        "value": rng.standard_normal((B, SQ, D), dtype=np.float32),
        "Wq": (rng.standard_normal((D, D), dtype=np.float32) * 0.02),
        "bq": np.zeros(D, np.float32),
        "Wk": (rng.standard_normal((D, D), dtype=np.float32) * 0.02),
        "bk": np.zeros(D, np.float32),
        "Wv": (rng.standard_normal((D, D), dtype=np.float32) * 0.02),
        "bv": np.zeros(D, np.float32),
        "Wo": (rng.standard_normal((D, D), dtype=np.float32) * 0.02),
        "bo": np.zeros(D, np.float32),
    }
    out = kernel(**ins)
    print("kernel ran, out shape", out.shape)
